# revision 10
# baseline (speedup 1.0000x reference)
"""Trainium2 Bass kernel for nn_Conv2d_NN_spatial (retrieval_knn).

Full-input contract: kernel(**inputs) takes the unsharded inputs and returns
the full output. Internally: data-parallel over batch across 8 NeuronCores
(4 batches per core).

Per-core algorithm:
  1. negd2 = 2*dot(x2, xs) - m2  via one 65-row-contraction fp32 matmul per
     128-token chunk (n2 term omitted: constant per token, rank-invariant).
  2. top-3 neighbors per token via DVE max (top-8) + max_index directly on
     the PSUM distance tile (tie-break == jax top_k).
  3. Projected tables P_k = W_k @ xs + bias/3 (o-permuted padded weights),
     spread into j-slices of R and folded 4->1 across partitions by a
     delta-matrix matmul: TABQ holds, per partition (b,oquad,tokhalf), rows
     [(k,m), 4 o-values] bf16 -- no table DMA.
  4. GPSIMD ap_gather with d=4 fetches 4 o-channels per index, 6144 indices
     per gpsimd core, SPLIT into 4 quarter-gathers pipelined against the
     distance/top-k loop (split-major processing order).  Idx streams
     round-trip DRAM in [lane][word] layout so fold DMAs stay coarse.
  5. Sum the 3 gathered projections per split (2 DVE strided adds) -> out.
Pixel unshuffle/shuffle are pure layout transforms done host-side.
"""
import numpy as np
import ml_dtypes

import concourse.bacc as bacc
import concourse.bass as bass
import concourse.mybir as mybir
import concourse.tile as tile
from concourse.bass_utils import run_bass_kernel_spmd

F32 = mybir.dt.float32
BF16 = mybir.dt.bfloat16
U16 = mybir.dt.uint16
I16 = mybir.dt.int16

N_CORES = 8
B_PER_CORE = 4
C1 = 64          # unshuffled channels
N = 4096         # tokens per batch (64*64)
NHALF = N // 2   # tokens per gather group (th = token half)
M = 256          # samples
K = 3
NIDX = NHALF * K  # gather indices per gpsimd core (6144)
NW = NIDX // 16   # idx words per lane (384)
NSPLIT = 4        # pipelined gather splits
CPS = 16 // NSPLIT  # chunks-per-half per split (4)
WPS = NW // NSPLIT  # idx words per split (96)
IPS = NIDX // NSPLIT  # idx per split per core (1536)

SIDX = [0, 4, 8, 13, 17, 21, 25, 29, 34, 38, 42, 46, 50, 55, 59, 63]
SAMPLE_FLAT = (np.array(SIDX)[:, None] * 64 + np.array(SIDX)[None, :]).reshape(-1)

_CACHE = {}


def build_program():
    """Build the per-core Bass program (SPMD: same program on all 8 cores)."""
    nc = bacc.Bacc("TRN2", target_bir_lowering=False, debug=False,
                   enable_asserts=False)

    x2e = nc.dram_tensor("x2e", [B_PER_CORE, 65, N], F32, kind="ExternalInput")
    xse = nc.dram_tensor("xse", [B_PER_CORE, 66, M], F32, kind="ExternalInput")
    wke = nc.dram_tensor("wke", [K, 66, 128], F32, kind="ExternalInput")
    # delta fold matrices: [b][o'' 128][psum col 128] bf16
    dle = nc.dram_tensor("dle", [B_PER_CORE, 128, 128], BF16,
                         kind="ExternalInput")
    outd = nc.dram_tensor("outd", [128, 2 * N], F32, kind="ExternalOutput")
    # idx stream scratch in [lane][word] layout:
    # stream step i = ((cp*8+pg)*3 + k)*16 + pl -> lane pl, word cp*24+pg*3+k
    idxscr = nc.dram_tensor("idxscr", [B_PER_CORE, 2, 16, NW], U16,
                            kind="Internal")

    AA = mybir.AluOpType

    with tile.TileContext(nc) as tc:
        with (
            tc.tile_pool(name="xp", bufs=3) as xp,
            tc.tile_pool(name="sp", bufs=4) as sp,
            tc.tile_pool(name="cst", bufs=1) as cst,
            tc.tile_pool(name="tbp", bufs=2) as tbp,
            tc.tile_pool(name="m8p", bufs=4) as m8p,
            tc.tile_pool(name="ixp", bufs=3) as ixp,
            tc.tile_pool(name="gp", bufs=1) as gp,
            tc.tile_pool(name="op", bufs=2) as op_,
            tc.tile_pool(name="ps", bufs=4, space=bass.MemorySpace.PSUM) as psp,
            tc.tile_pool(name="ps2", bufs=2, space=bass.MemorySpace.PSUM) as psp2,
            tc.tile_pool(name="ps3", bufs=2, space=bass.MemorySpace.PSUM) as psp3,
        ):
            onescol = cst.tile([64, 1], F32, tag="ones")
            nc.vector.memset(onescol[:], 1.0)
            wk = []
            for k in range(K):
                t = cst.tile([66, 128], F32, tag=f"wk{k}")
                nc.sync.dma_start(t[:], wke[k])
                wk.append(t)
            dl = []
            for b in range(B_PER_CORE):
                t = cst.tile([128, 128], BF16, tag=f"dl{b}")
                nc.sync.dma_start(t[:], dle[b])
                dl.append(t)

            # gather table: partition (b*32 + th*16 + oq), free (k, m, j)
            TABQ = gp.tile([128, K * M * 4], BF16, tag="TABQ")

            # ---- load S, compute m2 row, project + fold tables ----
            S_tiles = []
            for b in range(B_PER_CORE):
                S = sp.tile([66, M], F32, tag="S")
                nc.sync.dma_start(S[:], xse[b])
                SQ = sp.tile([64, M], F32, tag="SQ")
                nc.vector.tensor_tensor(SQ[:], S[0:64, :], S[0:64, :], op=AA.mult)
                m2ps = psp2.tile([128, M], F32, tag="tabps")
                nc.tensor.matmul(m2ps[64:65, :], onescol[:], SQ[:],
                                 tile_position=(0, 64))
                nc.scalar.activation(S[64:65, :], m2ps[64:65, :],
                                     mybir.ActivationFunctionType.Identity,
                                     bias=0.0, scale=-0.25)
                S_tiles.append(S)
                # P''-stack [128 o''=(j*32+oq), (k, m)] bf16
                PS = tbp.tile([128, K * M], BF16, tag="PS")
                for k in range(K):
                    tp = psp2.tile([128, M], F32, tag="tabps")
                    nc.tensor.matmul(tp[:], wk[k][:], S[:])
                    nc.scalar.copy(PS[:, k * M:(k + 1) * M], tp[:])
                # R[o'', (k, m, j)] = PS[o'', (k,m)] iff j == o''//32 else 0
                R = tbp.tile([128, K * M * 4], BF16, tag="R")
                nc.vector.memset(R[:], 0.0)
                rv = R[:].rearrange("p (r j) -> p r j", j=4)
                for j in range(4):
                    nc.scalar.copy(rv[j * 32:j * 32 + 16, :, j:j + 1],
                                   PS[j * 32:j * 32 + 16, :].rearrange(
                                       "p (r u) -> p r u", u=1))
                # fold 4->1 partitions: TABQ[b*32 + th*16 + oq] = P[oq*4+j]
                for ch in range(6):
                    tq = psp3.tile([128, 512], F32, tag="tqps")
                    nc.tensor.matmul(tq[:], dl[b][:],
                                     R[:, ch * 512:(ch + 1) * 512])
                    nc.scalar.copy(
                        TABQ[b * 32:(b + 1) * 32, ch * 512:(ch + 1) * 512],
                        tq[b * 32:(b + 1) * 32, :])

            # ---- distance + top-3, split-major for gather pipelining ----
            G = gp.tile([128, NIDX * 4], BF16, tag="G")
            WIDX = cst.tile([128, NW], I16, tag="WIDX")
            for s in range(NSPLIT):
                for b in range(B_PER_CORE):
                    # chunks of this split: cp in [CPS*s, CPS*(s+1)) per half
                    X = xp.tile([65, 2 * CPS * 128], F32, tag="X")
                    for th in range(2):
                        nc.sync.dma_start(
                            X[:, th * CPS * 128:(th + 1) * CPS * 128],
                            x2e[b][:, (th * 16 + CPS * s) * 128:
                                   (th * 16 + CPS * s + CPS) * 128])
                    S = S_tiles[b]
                    IDXS = ixp.tile([128, 2 * CPS * 8], U16, tag="IDXS")
                    for ci in range(2 * CPS):
                        nd = psp.tile([128, M], F32, tag="nd")
                        nc.tensor.matmul(nd[:], X[:, ci * 128:(ci + 1) * 128],
                                         S[0:65, :])
                        M8 = m8p.tile([128, 8], F32, tag="M8")
                        nc.vector.max(M8[:], nd[:])
                        nc.vector.max_index(IDXS[:, ci * 8:(ci + 1) * 8],
                                            M8[:], nd[:])

                    # slice k<3, add 256*k table-row offset
                    IDXC = ixp.tile([128, 2 * CPS * K], U16, tag="IDXC")
                    src = IDXS[:].rearrange("p (c e) -> p c e", e=8)
                    dst = IDXC[:].rearrange("p (c e) -> p c e", e=3)
                    for k in range(K):
                        nc.vector.tensor_scalar_add(dst[:, :, k:k + 1],
                                                    src[:, :, k:k + 1], 256 * k)
                    # fold out: DRAM[b][th][lane pl][word cp*24+pg*3+k]
                    srcv = IDXC[:].rearrange("p (th cp k) -> p th cp k",
                                             th=2, cp=CPS, k=K)
                    dstv = idxscr[b].rearrange(
                        "th pl (cp pg k) -> pl th cp pg k",
                        cp=16, pg=8, k=K)[:, :, CPS * s:CPS * (s + 1), :, :]
                    for pg in range(8):
                        for th in range(2):
                            nc.scalar.dma_start(
                                dstv[:, th, :, pg, :],
                                srcv[pg * 16:(pg + 1) * 16, th])

                # fold in: contiguous word-slice read per lane-partition
                for b in range(B_PER_CORE):
                    for th in range(2):
                        g = b * 2 + th
                        dst = WIDX[g * 16:(g + 1) * 16,
                                   WPS * s:WPS * (s + 1)].bitcast(U16)
                        nc.scalar.dma_start(
                            dst, idxscr[b, th][:, WPS * s:WPS * (s + 1)])

                # quarter-gather: all 4 batches, d=4 o-channels per index
                nc.gpsimd.ap_gather(
                    G[:, IPS * 4 * s:IPS * 4 * (s + 1)].rearrange(
                        "p (i j) -> p i j", j=4),
                    TABQ[:].rearrange("p (r j) -> p r j", j=4),
                    WIDX[:, WPS * s:WPS * (s + 1)],
                    channels=128, num_elems=K * M, d=4, num_idxs=IPS)

            # ---- k-sum per split: OUT[p,(cp,pg,pl,j)] = sum_k G[...] ----
            for s in range(NSPLIT):
                gv = G[:, IPS * 4 * s:IPS * 4 * (s + 1)].rearrange(
                    "p (cp pg k pl j) -> p (cp pg) k (pl j)",
                    cp=CPS, pg=8, k=K, pl=16, j=4)
                T1 = op_.tile([128, CPS * 8 * 64], F32, tag="T1")
                t1v = T1[:].rearrange("p (t w) -> p t w", w=64)
                OUT = op_.tile([128, CPS * 8 * 64], F32, tag="OUT")
                ov = OUT[:].rearrange("p (t w) -> p t w", w=64)
                nc.vector.tensor_tensor(t1v, gv[:, :, 0, :], gv[:, :, 1, :],
                                        op=AA.add)
                nc.vector.tensor_tensor(ov, t1v, gv[:, :, 2, :], op=AA.add)
                nc.sync.dma_start(
                    outd[:, CPS * 8 * 64 * s:CPS * 8 * 64 * (s + 1)], OUT[:])

    nc.compile()
    return nc


def host_prep(x, weight, bias):
    """Full inputs -> per-core in_maps (list of 8 dicts)."""
    x = np.ascontiguousarray(np.asarray(x), dtype=np.float32)
    weight = np.asarray(weight, dtype=np.float32)
    bias = np.asarray(bias, dtype=np.float32)
    B = x.shape[0]
    x1 = x.reshape(B, 16, 64, 2, 64, 2).transpose(0, 1, 3, 5, 2, 4)
    x2 = np.ascontiguousarray(x1).reshape(B, C1, N)
    xs = np.ascontiguousarray(x2[:, :, SAMPLE_FLAT])

    x2e = np.empty((B, 65, N), np.float32)
    x2e[:, :64] = x2
    x2e[:, 64] = 1.0
    xse = np.zeros((B, 66, M), np.float32)
    xse[:, :64] = xs * np.float32(2.0)
    xse[:, 65] = 1.0
    # padded o''-permutation: col o'' = j*32 + oq holds W row oq*4+j
    opp = np.arange(128)
    jj, oq = opp // 32, opp % 32
    valid = oq < 16
    orow = np.where(valid, (oq % 16) * 4 + jj, 0)
    wke = np.zeros((K, 66, 128), np.float32)
    for k in range(K):
        wke[k, :64] = np.where(valid[None, :],
                               weight[orow, :, k].T * np.float32(0.5), 0.0)
        wke[k, 65] = np.where(valid, bias[orow] * np.float32(1.0 / 3.0), 0.0)
    # delta fold: dle[b][o''][p] = (o''%32 == p%16) for p in b-block
    dle = np.zeros((B_PER_CORE, 128, 128), ml_dtypes.bfloat16)
    pp = np.arange(128)
    eq = (opp[:, None] % 32) == (pp[None, :] % 16)
    for b in range(B_PER_CORE):
        blk = (pp >= b * 32) & (pp < (b + 1) * 32)
        dle[b] = (eq & blk[None, :]).astype(ml_dtypes.bfloat16)

    in_maps = []
    for core in range(N_CORES):
        sl = slice(core * B_PER_CORE, (core + 1) * B_PER_CORE)
        in_maps.append({
            "x2e": np.ascontiguousarray(x2e[sl]),
            "xse": np.ascontiguousarray(xse[sl]),
            "wke": wke,
            "dle": dle,
        })
    return in_maps


def host_post(results):
    """Per-core outd [128, 8192] -> full output [32, 16, 128, 128]."""
    B = N_CORES * B_PER_CORE
    out = np.empty((B, C1, N), np.float32)
    for core in range(N_CORES):
        o = results[core]["outd"]  # [128, 8192] f32
        # partition = (b, th, oq); free = (cp, pg, pl, j); o_chan = oq*4+j
        arr = o.reshape(B_PER_CORE, 2, 16, 16, 8, 16, 4)  # b th oq cp pg pl j
        # out[b, oq*4+j, th*2048 + cp*128 + pg*16 + pl]
        blk = arr.transpose(0, 2, 6, 1, 3, 4, 5).reshape(B_PER_CORE, C1, N)
        out[core * B_PER_CORE:(core + 1) * B_PER_CORE] = blk
    out = out.reshape(B, C1, 64, 64)
    y = (out.reshape(B, 16, 2, 2, 64, 64).transpose(0, 1, 4, 2, 5, 3)
         .reshape(B, 16, 128, 128))
    return np.ascontiguousarray(y)


def kernel(x, weight, bias):
    if "nc" not in _CACHE:
        _CACHE["nc"] = build_program()
    nc = _CACHE["nc"]
    in_maps = host_prep(x, weight, bias)
    res = run_bass_kernel_spmd(nc, in_maps, core_ids=list(range(N_CORES)))
    return host_post(res.results)


# revision 12
# speedup vs baseline: 1.5575x; 1.5575x over previous
"""Trainium2 Bass kernel for nn_Conv2d_NN_spatial (retrieval_knn).

Full-input contract: kernel(**inputs) takes the unsharded inputs and returns
the full output. Internally: data-parallel over batch across 8 NeuronCores
(4 batches per core).

Per-core algorithm:
  1. negd2 = 2*dot(x2, xs) - m2  via one 65-row-contraction fp32 matmul per
     128-token chunk (n2 term omitted: constant per token, rank-invariant).
  2. top-3 neighbors per token via DVE max (top-8) + max_index directly on
     the PSUM distance tile (tie-break == jax top_k).
  3. Projected tables P_k = W_k @ xs + bias/3 (o-permuted padded weights),
     spread into j-slices of R and folded 4->1 across partitions by a
     delta-matrix matmul: TABQ holds, per partition (b,oquad,tokhalf), rows
     [(k,m), 4 o-values] bf16 -- no table DMA.
  4. GPSIMD ap_gather with d=4 fetches 4 o-channels per index, 6144 indices
     per gpsimd core, SPLIT into 4 quarter-gathers pipelined against the
     distance/top-k loop (split-major processing order).  Idx streams
     round-trip DRAM in [lane][word] layout so fold DMAs stay coarse.
  5. Sum the 3 gathered projections per split (2 DVE strided adds) -> out.
Pixel unshuffle/shuffle are pure layout transforms done host-side.
"""
import numpy as np
import ml_dtypes

import concourse.bacc as bacc
import concourse.bass as bass
import concourse.mybir as mybir
import concourse.tile as tile
from concourse.bass_utils import run_bass_kernel_spmd

F32 = mybir.dt.float32
BF16 = mybir.dt.bfloat16
U16 = mybir.dt.uint16
I16 = mybir.dt.int16

N_CORES = 8
B_PER_CORE = 4
C1 = 64          # unshuffled channels
N = 4096         # tokens per batch (64*64)
NHALF = N // 2   # tokens per gather group (th = token half)
M = 256          # samples
K = 3
NIDX = NHALF * K  # gather indices per gpsimd core (6144)
NW = NIDX // 16   # idx words per lane (384)
NSPLIT = 4        # pipelined gather splits
CPS = 16 // NSPLIT  # chunks-per-half per split (4)
WPS = NW // NSPLIT  # idx words per split (96)
IPS = NIDX // NSPLIT  # idx per split per core (1536)

SIDX = [0, 4, 8, 13, 17, 21, 25, 29, 34, 38, 42, 46, 50, 55, 59, 63]
SAMPLE_FLAT = (np.array(SIDX)[:, None] * 64 + np.array(SIDX)[None, :]).reshape(-1)

_CACHE = {}


def build_program():
    """Build the per-core Bass program (SPMD: same program on all 8 cores)."""
    nc = bacc.Bacc("TRN2", target_bir_lowering=False, debug=False,
                   enable_asserts=False)

    x2e = nc.dram_tensor("x2e", [B_PER_CORE, 65, N], F32, kind="ExternalInput")
    xse = nc.dram_tensor("xse", [B_PER_CORE, 66, M], F32, kind="ExternalInput")
    wke = nc.dram_tensor("wke", [K, 66, 128], F32, kind="ExternalInput")
    # delta fold matrices: [b][o'' 128][psum col 128] bf16
    dle = nc.dram_tensor("dle", [B_PER_CORE, 128, 128], BF16,
                         kind="ExternalInput")
    outd = nc.dram_tensor("outd", [128, 2 * N], F32, kind="ExternalOutput")
    # idx stream scratch in [lane][word] layout:
    # stream step i = ((cp*8+pg)*3 + k)*16 + pl -> lane pl, word cp*24+pg*3+k
    idxscr = nc.dram_tensor("idxscr", [NSPLIT, B_PER_CORE, 128, 2 * CPS * K],
                            U16, kind="Internal")

    AA = mybir.AluOpType

    with tile.TileContext(nc) as tc:
        with (
            tc.tile_pool(name="xp", bufs=3) as xp,
            tc.tile_pool(name="sp", bufs=4) as sp,
            tc.tile_pool(name="cst", bufs=1) as cst,
            tc.tile_pool(name="tbp", bufs=2) as tbp,
            tc.tile_pool(name="m8p", bufs=4) as m8p,
            tc.tile_pool(name="ixp", bufs=3) as ixp,
            tc.tile_pool(name="gp", bufs=1) as gp,
            tc.tile_pool(name="op", bufs=2) as op_,
            tc.tile_pool(name="ps", bufs=4, space=bass.MemorySpace.PSUM) as psp,
            tc.tile_pool(name="ps2", bufs=2, space=bass.MemorySpace.PSUM) as psp2,
            tc.tile_pool(name="ps3", bufs=2, space=bass.MemorySpace.PSUM) as psp3,
        ):
            onescol = cst.tile([64, 1], F32, tag="ones")
            nc.vector.memset(onescol[:], 1.0)
            wk = []
            for k in range(K):
                t = cst.tile([66, 128], F32, tag=f"wk{k}")
                nc.sync.dma_start(t[:], wke[k])
                wk.append(t)
            dl = []
            for b in range(B_PER_CORE):
                t = cst.tile([128, 128], BF16, tag=f"dl{b}")
                nc.sync.dma_start(t[:], dle[b])
                dl.append(t)

            # gather table: partition (b*32 + th*16 + oq), free (k, m, j)
            TABQ = gp.tile([128, K * M * 4], BF16, tag="TABQ")

            # ---- load S, compute m2 row, project + fold tables ----
            S_tiles = []
            for b in range(B_PER_CORE):
                S = sp.tile([66, M], F32, tag="S")
                nc.sync.dma_start(S[:], xse[b])
                SQ = sp.tile([64, M], F32, tag="SQ")
                nc.vector.tensor_tensor(SQ[:], S[0:64, :], S[0:64, :], op=AA.mult)
                m2ps = psp2.tile([128, M], F32, tag="tabps")
                nc.tensor.matmul(m2ps[64:65, :], onescol[:], SQ[:],
                                 tile_position=(0, 64))
                nc.scalar.activation(S[64:65, :], m2ps[64:65, :],
                                     mybir.ActivationFunctionType.Identity,
                                     bias=0.0, scale=-0.25)
                S_tiles.append(S)
                # P''-stack [128 o''=(j*32+oq), (k, m)] bf16
                PS = tbp.tile([128, K * M], BF16, tag="PS")
                for k in range(K):
                    tp = psp2.tile([128, M], F32, tag="tabps")
                    nc.tensor.matmul(tp[:], wk[k][:], S[:])
                    nc.scalar.copy(PS[:, k * M:(k + 1) * M], tp[:])
                # R[o'', (k, m, j)] = PS[o'', (k,m)] iff j == o''//32 else 0
                R = tbp.tile([128, K * M * 4], BF16, tag="R")
                nc.vector.memset(R[:], 0.0)
                rv = R[:].rearrange("p (r j) -> p r j", j=4)
                for j in range(4):
                    nc.scalar.copy(rv[j * 32:j * 32 + 16, :, j:j + 1],
                                   PS[j * 32:j * 32 + 16, :].rearrange(
                                       "p (r u) -> p r u", u=1))
                # fold 4->1 partitions: TABQ[b*32 + th*16 + oq] = P[oq*4+j]
                for ch in range(6):
                    tq = psp3.tile([128, 512], F32, tag="tqps")
                    nc.tensor.matmul(tq[:], dl[b][:],
                                     R[:, ch * 512:(ch + 1) * 512])
                    nc.scalar.copy(
                        TABQ[b * 32:(b + 1) * 32, ch * 512:(ch + 1) * 512],
                        tq[b * 32:(b + 1) * 32, :])

            # ---- distance + top-3, split-major for gather pipelining ----
            G = gp.tile([128, NIDX * 4], BF16, tag="G")
            WIDX = cst.tile([128, NW], I16, tag="WIDX")
            for s in range(NSPLIT):
                for b in range(B_PER_CORE):
                    # chunks of this split: cp in [CPS*s, CPS*(s+1)) per half
                    X = xp.tile([65, 2 * CPS * 128], F32, tag="X")
                    for th in range(2):
                        nc.sync.dma_start(
                            X[:, th * CPS * 128:(th + 1) * CPS * 128],
                            x2e[b][:, (th * 16 + CPS * s) * 128:
                                   (th * 16 + CPS * s + CPS) * 128])
                    S = S_tiles[b]
                    IDXS = ixp.tile([128, 2 * CPS * 8], U16, tag="IDXS")
                    for ci in range(2 * CPS):
                        nd = psp.tile([128, M], F32, tag="nd")
                        nc.tensor.matmul(nd[:], X[:, ci * 128:(ci + 1) * 128],
                                         S[0:65, :])
                        M8 = m8p.tile([128, 8], F32, tag="M8")
                        nc.vector.max(M8[:], nd[:])
                        nc.vector.max_index(IDXS[:, ci * 8:(ci + 1) * 8],
                                            M8[:], nd[:])

                    # slice k<3, add 256*k table-row offset
                    IDXC = ixp.tile([128, 2 * CPS * K], U16, tag="IDXC")
                    src = IDXS[:].rearrange("p (c e) -> p c e", e=8)
                    dst = IDXC[:].rearrange("p (c e) -> p c e", e=3)
                    for k in range(K):
                        nc.vector.tensor_scalar_add(dst[:, :, k:k + 1],
                                                    src[:, :, k:k + 1], 256 * k)
                    # fold hop 1: contiguous dump [128, 2*CPS*K] -> DRAM
                    nc.sync.dma_start(idxscr[s, b], IDXC[:])

                # fold hop 2: scramble on read into wrapped [lane][word]
                # word (pg, cp, k); WIDX[16g+q, f] = stream i = f*16 + q
                for b in range(B_PER_CORE):
                    srcw = idxscr[s, b].rearrange(
                        "(pg q) (th ck) -> q th pg ck",
                        pg=8, q=16, th=2, ck=CPS * K)
                    for th in range(2):
                        g = b * 2 + th
                        dst = WIDX[g * 16:(g + 1) * 16,
                                   WPS * s:WPS * (s + 1)].bitcast(U16).rearrange(
                            "q (pg ck) -> q pg ck", pg=8, ck=CPS * K)
                        nc.scalar.dma_start(dst, srcw[:, th])

                # quarter-gather: all 4 batches, d=4 o-channels per index
                nc.gpsimd.ap_gather(
                    G[:, IPS * 4 * s:IPS * 4 * (s + 1)].rearrange(
                        "p (i j) -> p i j", j=4),
                    TABQ[:].rearrange("p (r j) -> p r j", j=4),
                    WIDX[:, WPS * s:WPS * (s + 1)],
                    channels=128, num_elems=K * M, d=4, num_idxs=IPS)

            # ---- k-sum per split: OUT[p,(cp,pg,pl,j)] = sum_k G[...] ----
            for s in range(NSPLIT):
                gv = G[:, IPS * 4 * s:IPS * 4 * (s + 1)].rearrange(
                    "p (pg cp k pl j) -> p (pg cp) k (pl j)",
                    pg=8, cp=CPS, k=K, pl=16, j=4)
                T1 = op_.tile([128, CPS * 8 * 64], F32, tag="T1")
                t1v = T1[:].rearrange("p (t w) -> p t w", w=64)
                OUT = op_.tile([128, CPS * 8 * 64], F32, tag="OUT")
                ov = OUT[:].rearrange("p (t w) -> p t w", w=64)
                nc.vector.tensor_tensor(t1v, gv[:, :, 0, :], gv[:, :, 1, :],
                                        op=AA.add)
                nc.vector.tensor_tensor(ov, t1v, gv[:, :, 2, :], op=AA.add)
                nc.sync.dma_start(
                    outd[:, CPS * 8 * 64 * s:CPS * 8 * 64 * (s + 1)], OUT[:])

    nc.compile()
    return nc


def host_prep(x, weight, bias):
    """Full inputs -> per-core in_maps (list of 8 dicts)."""
    x = np.ascontiguousarray(np.asarray(x), dtype=np.float32)
    weight = np.asarray(weight, dtype=np.float32)
    bias = np.asarray(bias, dtype=np.float32)
    B = x.shape[0]
    x1 = x.reshape(B, 16, 64, 2, 64, 2).transpose(0, 1, 3, 5, 2, 4)
    x2 = np.ascontiguousarray(x1).reshape(B, C1, N)
    xs = np.ascontiguousarray(x2[:, :, SAMPLE_FLAT])

    x2e = np.empty((B, 65, N), np.float32)
    x2e[:, :64] = x2
    x2e[:, 64] = 1.0
    xse = np.zeros((B, 66, M), np.float32)
    xse[:, :64] = xs * np.float32(2.0)
    xse[:, 65] = 1.0
    # padded o''-permutation: col o'' = j*32 + oq holds W row oq*4+j
    opp = np.arange(128)
    jj, oq = opp // 32, opp % 32
    valid = oq < 16
    orow = np.where(valid, (oq % 16) * 4 + jj, 0)
    wke = np.zeros((K, 66, 128), np.float32)
    for k in range(K):
        wke[k, :64] = np.where(valid[None, :],
                               weight[orow, :, k].T * np.float32(0.5), 0.0)
        wke[k, 65] = np.where(valid, bias[orow] * np.float32(1.0 / 3.0), 0.0)
    # delta fold: dle[b][o''][p] = (o''%32 == p%16) for p in b-block
    dle = np.zeros((B_PER_CORE, 128, 128), ml_dtypes.bfloat16)
    pp = np.arange(128)
    eq = (opp[:, None] % 32) == (pp[None, :] % 16)
    for b in range(B_PER_CORE):
        blk = (pp >= b * 32) & (pp < (b + 1) * 32)
        dle[b] = (eq & blk[None, :]).astype(ml_dtypes.bfloat16)

    in_maps = []
    for core in range(N_CORES):
        sl = slice(core * B_PER_CORE, (core + 1) * B_PER_CORE)
        in_maps.append({
            "x2e": np.ascontiguousarray(x2e[sl]),
            "xse": np.ascontiguousarray(xse[sl]),
            "wke": wke,
            "dle": dle,
        })
    return in_maps


def host_post(results):
    """Per-core outd [128, 8192] -> full output [32, 16, 128, 128]."""
    B = N_CORES * B_PER_CORE
    out = np.empty((B, C1, N), np.float32)
    for core in range(N_CORES):
        o = results[core]["outd"]  # [128, 8192] f32
        # partition = (b, th, oq); free = (s, pg, cp, pl, j); o = oq*4+j
        arr = o.reshape(B_PER_CORE, 2, 16, NSPLIT, 8, CPS, 16, 4)
        # out[b, oq*4+j, th*2048 + (s*CPS+cp)*128 + pg*16 + pl]
        blk = (arr.transpose(0, 2, 7, 1, 3, 5, 4, 6)  # b oq j th s cp pg pl
               .reshape(B_PER_CORE, C1, N))
        out[core * B_PER_CORE:(core + 1) * B_PER_CORE] = blk
    out = out.reshape(B, C1, 64, 64)
    y = (out.reshape(B, 16, 2, 2, 64, 64).transpose(0, 1, 4, 2, 5, 3)
         .reshape(B, 16, 128, 128))
    return np.ascontiguousarray(y)


def kernel(x, weight, bias):
    if "nc" not in _CACHE:
        _CACHE["nc"] = build_program()
    nc = _CACHE["nc"]
    in_maps = host_prep(x, weight, bias)
    res = run_bass_kernel_spmd(nc, in_maps, core_ids=list(range(N_CORES)))
    return host_post(res.results)


# revision 13
# speedup vs baseline: 1.6027x; 1.0290x over previous
"""Trainium2 Bass kernel for nn_Conv2d_NN_spatial (retrieval_knn).

Full-input contract: kernel(**inputs) takes the unsharded inputs and returns
the full output. Internally: data-parallel over batch across 8 NeuronCores
(4 batches per core).

Per-core algorithm:
  1. negd2 = 2*dot(x2, xs) - m2  via one 65-row-contraction fp32 matmul per
     128-token chunk (n2 term omitted: constant per token, rank-invariant).
  2. top-3 neighbors per token via DVE max (top-8) + max_index directly on
     the PSUM distance tile (tie-break == jax top_k).
  3. Projected tables P_k = W_k @ xs + bias/3 (o-permuted padded weights),
     spread into j-slices of R and folded 4->1 across partitions by a
     delta-matrix matmul: TABQ holds, per partition (b,oquad,tokhalf), rows
     [(k,m), 4 o-values] bf16 -- no table DMA.
  4. GPSIMD ap_gather with d=4 fetches 4 o-channels per index, 6144 indices
     per gpsimd core, SPLIT into 4 quarter-gathers pipelined against the
     distance/top-k loop (split-major processing order).  Idx streams
     round-trip DRAM in [lane][word] layout so fold DMAs stay coarse.
  5. Sum the 3 gathered projections per split (2 DVE strided adds) -> out.
Pixel unshuffle/shuffle are pure layout transforms done host-side.
"""
import numpy as np
import ml_dtypes

import concourse.bacc as bacc
import concourse.bass as bass
import concourse.mybir as mybir
import concourse.tile as tile
from concourse.bass_utils import run_bass_kernel_spmd

F32 = mybir.dt.float32
BF16 = mybir.dt.bfloat16
U16 = mybir.dt.uint16
I16 = mybir.dt.int16

N_CORES = 8
B_PER_CORE = 4
C1 = 64          # unshuffled channels
N = 4096         # tokens per batch (64*64)
NHALF = N // 2   # tokens per gather group (th = token half)
M = 256          # samples
K = 3
NIDX = NHALF * K  # gather indices per gpsimd core (6144)
NW = NIDX // 16   # idx words per lane (384)
NSPLIT = 4        # pipelined gather splits
CPS = 16 // NSPLIT  # chunks-per-half per split (4)
WPS = NW // NSPLIT  # idx words per split (96)
IPS = NIDX // NSPLIT  # idx per split per core (1536)

SIDX = [0, 4, 8, 13, 17, 21, 25, 29, 34, 38, 42, 46, 50, 55, 59, 63]
SAMPLE_FLAT = (np.array(SIDX)[:, None] * 64 + np.array(SIDX)[None, :]).reshape(-1)

_CACHE = {}


def build_program():
    """Build the per-core Bass program (SPMD: same program on all 8 cores)."""
    nc = bacc.Bacc("TRN2", target_bir_lowering=False, debug=False,
                   enable_asserts=False)

    x2e = nc.dram_tensor("x2e", [B_PER_CORE, 65, N], F32, kind="ExternalInput")
    xse = nc.dram_tensor("xse", [B_PER_CORE, 66, M], F32, kind="ExternalInput")
    wke = nc.dram_tensor("wke", [K, 66, 128], F32, kind="ExternalInput")
    # delta fold matrices: [b][o'' 128][psum col 128] bf16
    dle = nc.dram_tensor("dle", [B_PER_CORE, 128, 128], BF16,
                         kind="ExternalInput")
    outd = nc.dram_tensor("outd", [128, 2 * N], F32, kind="ExternalOutput")
    # idx stream scratch in [lane][word] layout:
    # stream step i = ((cp*8+pg)*3 + k)*16 + pl -> lane pl, word cp*24+pg*3+k
    idxscr = nc.dram_tensor("idxscr", [NSPLIT, B_PER_CORE, 128, 2 * CPS * K],
                            U16, kind="Internal")

    AA = mybir.AluOpType

    with tile.TileContext(nc) as tc:
        with (
            tc.tile_pool(name="xp", bufs=3) as xp,
            tc.tile_pool(name="sp", bufs=4) as sp,
            tc.tile_pool(name="cst", bufs=1) as cst,
            tc.tile_pool(name="tbp", bufs=2) as tbp,
            tc.tile_pool(name="m8p", bufs=4) as m8p,
            tc.tile_pool(name="ixp", bufs=3) as ixp,
            tc.tile_pool(name="gp", bufs=1) as gp,
            tc.tile_pool(name="op", bufs=2) as op_,
            tc.tile_pool(name="ps", bufs=4, space=bass.MemorySpace.PSUM) as psp,
            tc.tile_pool(name="ps2", bufs=2, space=bass.MemorySpace.PSUM) as psp2,
            tc.tile_pool(name="ps3", bufs=2, space=bass.MemorySpace.PSUM) as psp3,
        ):
            onescol = cst.tile([64, 1], F32, tag="ones")
            nc.vector.memset(onescol[:], 1.0)
            wk = []
            for k in range(K):
                t = cst.tile([66, 128], F32, tag=f"wk{k}")
                nc.sync.dma_start(t[:], wke[k])
                wk.append(t)
            dl = []
            for b in range(B_PER_CORE):
                t = cst.tile([128, 128], BF16, tag=f"dl{b}")
                nc.sync.dma_start(t[:], dle[b])
                dl.append(t)

            # gather table: partition (b*32 + th*16 + oq), free (k, m, j)
            TABQ = gp.tile([128, K * M * 4], BF16, tag="TABQ")

            # ---- load S, compute m2 row, project + fold tables ----
            S_tiles = []
            for b in range(B_PER_CORE):
                S = sp.tile([66, M], F32, tag="S")
                nc.sync.dma_start(S[:], xse[b])
                SQ = sp.tile([64, M], F32, tag="SQ")
                nc.vector.tensor_tensor(SQ[:], S[0:64, :], S[0:64, :], op=AA.mult)
                m2ps = psp2.tile([128, M], F32, tag="tabps")
                nc.tensor.matmul(m2ps[64:65, :], onescol[:], SQ[:],
                                 tile_position=(0, 64))
                nc.scalar.activation(S[64:65, :], m2ps[64:65, :],
                                     mybir.ActivationFunctionType.Identity,
                                     bias=0.0, scale=-0.25)
                S_tiles.append(S)
                # P''-stack [128 o''=(j*32+oq), (k, m)] bf16
                PS = tbp.tile([128, K * M], BF16, tag="PS")
                for k in range(K):
                    tp = psp2.tile([128, M], F32, tag="tabps")
                    nc.tensor.matmul(tp[:], wk[k][:], S[:])
                    nc.scalar.copy(PS[:, k * M:(k + 1) * M], tp[:])
                # R[o'', (k, m, j)] = PS[o'', (k,m)] iff j == o''//32 else 0
                R = tbp.tile([128, K * M * 4], BF16, tag="R")
                nc.vector.memset(R[:], 0.0)
                rv = R[:].rearrange("p (r j) -> p r j", j=4)
                for j in range(4):
                    nc.vector.tensor_scalar_add(
                        rv[j * 32:j * 32 + 16, :, j:j + 1],
                        PS[j * 32:j * 32 + 16, :].rearrange(
                            "p (r u) -> p r u", u=1), 0.0)
                # fold 4->1 partitions: TABQ[b*32 + th*16 + oq] = P[oq*4+j]
                for ch in range(6):
                    tq = psp3.tile([128, 512], F32, tag="tqps")
                    nc.tensor.matmul(tq[:], dl[b][:],
                                     R[:, ch * 512:(ch + 1) * 512])
                    nc.scalar.copy(
                        TABQ[b * 32:(b + 1) * 32, ch * 512:(ch + 1) * 512],
                        tq[b * 32:(b + 1) * 32, :])

            # ---- distance + top-3, split-major for gather pipelining ----
            G = gp.tile([128, NIDX * 4], BF16, tag="G")
            WIDX = cst.tile([128, NW], I16, tag="WIDX")
            for s in range(NSPLIT):
                for b in range(B_PER_CORE):
                    # chunks of this split: cp in [CPS*s, CPS*(s+1)) per half
                    X = xp.tile([65, 2 * CPS * 128], F32, tag="X")
                    for th in range(2):
                        nc.sync.dma_start(
                            X[:, th * CPS * 128:(th + 1) * CPS * 128],
                            x2e[b][:, (th * 16 + CPS * s) * 128:
                                   (th * 16 + CPS * s + CPS) * 128])
                    S = S_tiles[b]
                    IDXS = ixp.tile([128, 2 * CPS * 8], U16, tag="IDXS")
                    for ci in range(2 * CPS):
                        nd = psp.tile([128, M], F32, tag="nd")
                        nc.tensor.matmul(nd[:], X[:, ci * 128:(ci + 1) * 128],
                                         S[0:65, :])
                        M8 = m8p.tile([128, 8], F32, tag="M8")
                        nc.vector.max(M8[:], nd[:])
                        nc.vector.max_index(IDXS[:, ci * 8:(ci + 1) * 8],
                                            M8[:], nd[:])

                    # slice k<3, add 256*k table-row offset
                    IDXC = ixp.tile([128, 2 * CPS * K], U16, tag="IDXC")
                    src = IDXS[:].rearrange("p (c e) -> p c e", e=8)
                    dst = IDXC[:].rearrange("p (c e) -> p c e", e=3)
                    for k in range(K):
                        nc.vector.tensor_scalar_add(dst[:, :, k:k + 1],
                                                    src[:, :, k:k + 1], 256 * k)
                    # fold hop 1: contiguous dump [128, 2*CPS*K] -> DRAM
                    nc.sync.dma_start(idxscr[s, b], IDXC[:])

                # fold hop 2: scramble on read into wrapped [lane][word]
                # word (pg, cp, k); WIDX[16g+q, f] = stream i = f*16 + q
                for b in range(B_PER_CORE):
                    srcw = idxscr[s, b].rearrange(
                        "(pg q) (th ck) -> q th pg ck",
                        pg=8, q=16, th=2, ck=CPS * K)
                    for th in range(2):
                        g = b * 2 + th
                        dst = WIDX[g * 16:(g + 1) * 16,
                                   WPS * s:WPS * (s + 1)].bitcast(U16).rearrange(
                            "q (pg ck) -> q pg ck", pg=8, ck=CPS * K)
                        nc.scalar.dma_start(dst, srcw[:, th])

                # quarter-gather: all 4 batches, d=4 o-channels per index
                nc.gpsimd.ap_gather(
                    G[:, IPS * 4 * s:IPS * 4 * (s + 1)].rearrange(
                        "p (i j) -> p i j", j=4),
                    TABQ[:].rearrange("p (r j) -> p r j", j=4),
                    WIDX[:, WPS * s:WPS * (s + 1)],
                    channels=128, num_elems=K * M, d=4, num_idxs=IPS)

            # ---- k-sum per split: OUT[p,(cp,pg,pl,j)] = sum_k G[...] ----
            for s in range(NSPLIT):
                gv = G[:, IPS * 4 * s:IPS * 4 * (s + 1)].rearrange(
                    "p (pg cp k pl j) -> p (pg cp) k (pl j)",
                    pg=8, cp=CPS, k=K, pl=16, j=4)
                T1 = op_.tile([128, CPS * 8 * 64], F32, tag="T1")
                t1v = T1[:].rearrange("p (t w) -> p t w", w=64)
                OUT = op_.tile([128, CPS * 8 * 64], F32, tag="OUT")
                ov = OUT[:].rearrange("p (t w) -> p t w", w=64)
                nc.vector.tensor_tensor(t1v, gv[:, :, 0, :], gv[:, :, 1, :],
                                        op=AA.add)
                nc.vector.tensor_tensor(ov, t1v, gv[:, :, 2, :], op=AA.add)
                nc.sync.dma_start(
                    outd[:, CPS * 8 * 64 * s:CPS * 8 * 64 * (s + 1)], OUT[:])

    nc.compile()
    return nc


def host_prep(x, weight, bias):
    """Full inputs -> per-core in_maps (list of 8 dicts)."""
    x = np.ascontiguousarray(np.asarray(x), dtype=np.float32)
    weight = np.asarray(weight, dtype=np.float32)
    bias = np.asarray(bias, dtype=np.float32)
    B = x.shape[0]
    x1 = x.reshape(B, 16, 64, 2, 64, 2).transpose(0, 1, 3, 5, 2, 4)
    x2 = np.ascontiguousarray(x1).reshape(B, C1, N)
    xs = np.ascontiguousarray(x2[:, :, SAMPLE_FLAT])

    x2e = np.empty((B, 65, N), np.float32)
    x2e[:, :64] = x2
    x2e[:, 64] = 1.0
    xse = np.zeros((B, 66, M), np.float32)
    xse[:, :64] = xs * np.float32(2.0)
    xse[:, 65] = 1.0
    # padded o''-permutation: col o'' = j*32 + oq holds W row oq*4+j
    opp = np.arange(128)
    jj, oq = opp // 32, opp % 32
    valid = oq < 16
    orow = np.where(valid, (oq % 16) * 4 + jj, 0)
    wke = np.zeros((K, 66, 128), np.float32)
    for k in range(K):
        wke[k, :64] = np.where(valid[None, :],
                               weight[orow, :, k].T * np.float32(0.5), 0.0)
        wke[k, 65] = np.where(valid, bias[orow] * np.float32(1.0 / 3.0), 0.0)
    # delta fold: dle[b][o''][p] = (o''%32 == p%16) for p in b-block
    dle = np.zeros((B_PER_CORE, 128, 128), ml_dtypes.bfloat16)
    pp = np.arange(128)
    eq = (opp[:, None] % 32) == (pp[None, :] % 16)
    for b in range(B_PER_CORE):
        blk = (pp >= b * 32) & (pp < (b + 1) * 32)
        dle[b] = (eq & blk[None, :]).astype(ml_dtypes.bfloat16)

    in_maps = []
    for core in range(N_CORES):
        sl = slice(core * B_PER_CORE, (core + 1) * B_PER_CORE)
        in_maps.append({
            "x2e": np.ascontiguousarray(x2e[sl]),
            "xse": np.ascontiguousarray(xse[sl]),
            "wke": wke,
            "dle": dle,
        })
    return in_maps


def host_post(results):
    """Per-core outd [128, 8192] -> full output [32, 16, 128, 128]."""
    B = N_CORES * B_PER_CORE
    out = np.empty((B, C1, N), np.float32)
    for core in range(N_CORES):
        o = results[core]["outd"]  # [128, 8192] f32
        # partition = (b, th, oq); free = (s, pg, cp, pl, j); o = oq*4+j
        arr = o.reshape(B_PER_CORE, 2, 16, NSPLIT, 8, CPS, 16, 4)
        # out[b, oq*4+j, th*2048 + (s*CPS+cp)*128 + pg*16 + pl]
        blk = (arr.transpose(0, 2, 7, 1, 3, 5, 4, 6)  # b oq j th s cp pg pl
               .reshape(B_PER_CORE, C1, N))
        out[core * B_PER_CORE:(core + 1) * B_PER_CORE] = blk
    out = out.reshape(B, C1, 64, 64)
    y = (out.reshape(B, 16, 2, 2, 64, 64).transpose(0, 1, 4, 2, 5, 3)
         .reshape(B, 16, 128, 128))
    return np.ascontiguousarray(y)


def kernel(x, weight, bias):
    if "nc" not in _CACHE:
        _CACHE["nc"] = build_program()
    nc = _CACHE["nc"]
    in_maps = host_prep(x, weight, bias)
    res = run_bass_kernel_spmd(nc, in_maps, core_ids=list(range(N_CORES)))
    return host_post(res.results)


# revision 14
# speedup vs baseline: 1.6588x; 1.0350x over previous
"""Trainium2 Bass kernel for nn_Conv2d_NN_spatial (retrieval_knn).

Full-input contract: kernel(**inputs) takes the unsharded inputs and returns
the full output. Internally: data-parallel over batch across 8 NeuronCores
(4 batches per core).

Per-core algorithm:
  1. negd2 = 2*dot(x2, xs) - m2  via one 65-row-contraction fp32 matmul per
     128-token chunk (n2 term omitted: constant per token, rank-invariant).
  2. top-3 neighbors per token via DVE max (top-8) + max_index directly on
     the PSUM distance tile (tie-break == jax top_k).
  3. Projected tables P_k = W_k @ xs + bias/3 (o-permuted padded weights),
     spread into j-slices of R and folded 4->1 across partitions by a
     delta-matrix matmul: TABQ holds, per partition (b,oquad,tokhalf), rows
     [(k,m), 4 o-values] bf16 -- no table DMA.
  4. GPSIMD ap_gather with d=4 fetches 4 o-channels per index, 6144 indices
     per gpsimd core, SPLIT into 4 quarter-gathers pipelined against the
     distance/top-k loop (split-major processing order).  Idx streams
     round-trip DRAM in [lane][word] layout so fold DMAs stay coarse.
  5. Sum the 3 gathered projections per split (2 DVE strided adds) -> out.
Pixel unshuffle/shuffle are pure layout transforms done host-side.
"""
import numpy as np
import ml_dtypes

import concourse.bacc as bacc
import concourse.bass as bass
import concourse.mybir as mybir
import concourse.tile as tile
from concourse.bass_utils import run_bass_kernel_spmd

F32 = mybir.dt.float32
BF16 = mybir.dt.bfloat16
U16 = mybir.dt.uint16
I16 = mybir.dt.int16

N_CORES = 8
B_PER_CORE = 4
C1 = 64          # unshuffled channels
N = 4096         # tokens per batch (64*64)
NHALF = N // 2   # tokens per gather group (th = token half)
M = 256          # samples
K = 3
NIDX = NHALF * K  # gather indices per gpsimd core (6144)
NW = NIDX // 16   # idx words per lane (384)
NSPLIT = 8        # pipelined gather splits
CPS = 16 // NSPLIT  # chunks-per-half per split (4)
WPS = NW // NSPLIT  # idx words per split (96)
IPS = NIDX // NSPLIT  # idx per split per core (1536)

SIDX = [0, 4, 8, 13, 17, 21, 25, 29, 34, 38, 42, 46, 50, 55, 59, 63]
SAMPLE_FLAT = (np.array(SIDX)[:, None] * 64 + np.array(SIDX)[None, :]).reshape(-1)

_CACHE = {}


def build_program():
    """Build the per-core Bass program (SPMD: same program on all 8 cores)."""
    nc = bacc.Bacc("TRN2", target_bir_lowering=False, debug=False,
                   enable_asserts=False)

    x2e = nc.dram_tensor("x2e", [B_PER_CORE, 65, N], F32, kind="ExternalInput")
    xse = nc.dram_tensor("xse", [B_PER_CORE, 66, M], F32, kind="ExternalInput")
    wke = nc.dram_tensor("wke", [K, 66, 128], F32, kind="ExternalInput")
    # delta fold matrices: [b][o'' 128][psum col 128] bf16
    dle = nc.dram_tensor("dle", [B_PER_CORE, 128, 128], BF16,
                         kind="ExternalInput")
    outd = nc.dram_tensor("outd", [128, 2 * N], F32, kind="ExternalOutput")
    # idx stream scratch in [lane][word] layout:
    # stream step i = ((cp*8+pg)*3 + k)*16 + pl -> lane pl, word cp*24+pg*3+k
    idxscr = nc.dram_tensor("idxscr", [NSPLIT, B_PER_CORE, 128, 2 * CPS * K],
                            U16, kind="Internal")

    AA = mybir.AluOpType

    with tile.TileContext(nc) as tc:
        with (
            tc.tile_pool(name="xp", bufs=3) as xp,
            tc.tile_pool(name="sp", bufs=4) as sp,
            tc.tile_pool(name="cst", bufs=1) as cst,
            tc.tile_pool(name="tbp", bufs=2) as tbp,
            tc.tile_pool(name="m8p", bufs=4) as m8p,
            tc.tile_pool(name="ixp", bufs=3) as ixp,
            tc.tile_pool(name="gp", bufs=1) as gp,
            tc.tile_pool(name="op", bufs=2) as op_,
            tc.tile_pool(name="ps", bufs=4, space=bass.MemorySpace.PSUM) as psp,
            tc.tile_pool(name="ps2", bufs=2, space=bass.MemorySpace.PSUM) as psp2,
            tc.tile_pool(name="ps3", bufs=2, space=bass.MemorySpace.PSUM) as psp3,
        ):
            onescol = cst.tile([64, 1], F32, tag="ones")
            nc.vector.memset(onescol[:], 1.0)
            wk = []
            for k in range(K):
                t = cst.tile([66, 128], F32, tag=f"wk{k}")
                nc.sync.dma_start(t[:], wke[k])
                wk.append(t)
            dl = []
            for b in range(B_PER_CORE):
                t = cst.tile([128, 128], BF16, tag=f"dl{b}")
                nc.sync.dma_start(t[:], dle[b])
                dl.append(t)

            # gather table: partition (b*32 + th*16 + oq), free (k, m, j)
            TABQ = gp.tile([128, K * M * 4], BF16, tag="TABQ")

            # ---- load S, compute m2 row, project + fold tables ----
            S_tiles = []
            for b in range(B_PER_CORE):
                S = sp.tile([66, M], F32, tag="S")
                nc.sync.dma_start(S[:], xse[b])
                SQ = sp.tile([64, M], F32, tag="SQ")
                nc.vector.tensor_tensor(SQ[:], S[0:64, :], S[0:64, :], op=AA.mult)
                m2ps = psp2.tile([128, M], F32, tag="tabps")
                nc.tensor.matmul(m2ps[64:65, :], onescol[:], SQ[:],
                                 tile_position=(0, 64))
                nc.scalar.activation(S[64:65, :], m2ps[64:65, :],
                                     mybir.ActivationFunctionType.Identity,
                                     bias=0.0, scale=-0.25)
                S_tiles.append(S)
                # P''-stack [128 o''=(j*32+oq), (k, m)] bf16
                PS = tbp.tile([128, K * M], BF16, tag="PS")
                for k in range(K):
                    tp = psp2.tile([128, M], F32, tag="tabps")
                    nc.tensor.matmul(tp[:], wk[k][:], S[:])
                    nc.scalar.copy(PS[:, k * M:(k + 1) * M], tp[:])
                # R[o'', (k, m, j)] = PS[o'', (k,m)] iff j == o''//32 else 0
                R = tbp.tile([128, K * M * 4], BF16, tag="R")
                nc.vector.memset(R[:], 0.0)
                rv = R[:].rearrange("p (r j) -> p r j", j=4)
                for j in range(4):
                    nc.vector.tensor_scalar_add(
                        rv[j * 32:j * 32 + 16, :, j:j + 1],
                        PS[j * 32:j * 32 + 16, :].rearrange(
                            "p (r u) -> p r u", u=1), 0.0)
                # fold 4->1 partitions: TABQ[b*32 + th*16 + oq] = P[oq*4+j]
                for ch in range(6):
                    tq = psp3.tile([128, 512], F32, tag="tqps")
                    nc.tensor.matmul(tq[:], dl[b][:],
                                     R[:, ch * 512:(ch + 1) * 512])
                    nc.scalar.copy(
                        TABQ[b * 32:(b + 1) * 32, ch * 512:(ch + 1) * 512],
                        tq[b * 32:(b + 1) * 32, :])

            # ---- distance + top-3, split-major for gather pipelining ----
            G = gp.tile([128, NIDX * 4], BF16, tag="G")
            WIDX = cst.tile([128, NW], I16, tag="WIDX")
            for s in range(NSPLIT):
                for b in range(B_PER_CORE):
                    # chunks of this split: cp in [CPS*s, CPS*(s+1)) per half
                    X = xp.tile([65, 2 * CPS * 128], F32, tag="X")
                    for th in range(2):
                        nc.sync.dma_start(
                            X[:, th * CPS * 128:(th + 1) * CPS * 128],
                            x2e[b][:, (th * 16 + CPS * s) * 128:
                                   (th * 16 + CPS * s + CPS) * 128])
                    S = S_tiles[b]
                    IDXS = ixp.tile([128, 2 * CPS * 8], U16, tag="IDXS")
                    for ci in range(2 * CPS):
                        nd = psp.tile([128, M], F32, tag="nd")
                        nc.tensor.matmul(nd[:], X[:, ci * 128:(ci + 1) * 128],
                                         S[0:65, :])
                        M8 = m8p.tile([128, 8], F32, tag="M8")
                        nc.vector.max(M8[:], nd[:])
                        nc.vector.max_index(IDXS[:, ci * 8:(ci + 1) * 8],
                                            M8[:], nd[:])

                    # slice k<3, add 256*k table-row offset
                    IDXC = ixp.tile([128, 2 * CPS * K], U16, tag="IDXC")
                    src = IDXS[:].rearrange("p (c e) -> p c e", e=8)
                    dst = IDXC[:].rearrange("p (c e) -> p c e", e=3)
                    for k in range(K):
                        nc.vector.tensor_scalar_add(dst[:, :, k:k + 1],
                                                    src[:, :, k:k + 1], 256 * k)
                    # fold hop 1: contiguous dump [128, 2*CPS*K] -> DRAM
                    nc.scalar.dma_start(idxscr[s, b], IDXC[:])

                # fold hop 2: scramble on read into wrapped [lane][word]
                # word (pg, cp, k); WIDX[16g+q, f] = stream i = f*16 + q
                for b in range(B_PER_CORE):
                    srcw = idxscr[s, b].rearrange(
                        "(pg q) (th ck) -> q th pg ck",
                        pg=8, q=16, th=2, ck=CPS * K)
                    for th in range(2):
                        g = b * 2 + th
                        dst = WIDX[g * 16:(g + 1) * 16,
                                   WPS * s:WPS * (s + 1)].bitcast(U16).rearrange(
                            "q (pg ck) -> q pg ck", pg=8, ck=CPS * K)
                        nc.scalar.dma_start(dst, srcw[:, th])

                # quarter-gather: all 4 batches, d=4 o-channels per index
                nc.gpsimd.ap_gather(
                    G[:, IPS * 4 * s:IPS * 4 * (s + 1)].rearrange(
                        "p (i j) -> p i j", j=4),
                    TABQ[:].rearrange("p (r j) -> p r j", j=4),
                    WIDX[:, WPS * s:WPS * (s + 1)],
                    channels=128, num_elems=K * M, d=4, num_idxs=IPS)

            # ---- k-sum per split: OUT[p,(cp,pg,pl,j)] = sum_k G[...] ----
            for s in range(NSPLIT):
                gv = G[:, IPS * 4 * s:IPS * 4 * (s + 1)].rearrange(
                    "p (pg cp k pl j) -> p (pg cp) k (pl j)",
                    pg=8, cp=CPS, k=K, pl=16, j=4)
                T1 = op_.tile([128, CPS * 8 * 64], F32, tag="T1")
                t1v = T1[:].rearrange("p (t w) -> p t w", w=64)
                OUT = op_.tile([128, CPS * 8 * 64], F32, tag="OUT")
                ov = OUT[:].rearrange("p (t w) -> p t w", w=64)
                nc.vector.tensor_tensor(t1v, gv[:, :, 0, :], gv[:, :, 1, :],
                                        op=AA.add)
                nc.vector.tensor_tensor(ov, t1v, gv[:, :, 2, :], op=AA.add)
                nc.sync.dma_start(
                    outd[:, CPS * 8 * 64 * s:CPS * 8 * 64 * (s + 1)], OUT[:])

    nc.compile()
    return nc


def host_prep(x, weight, bias):
    """Full inputs -> per-core in_maps (list of 8 dicts)."""
    x = np.ascontiguousarray(np.asarray(x), dtype=np.float32)
    weight = np.asarray(weight, dtype=np.float32)
    bias = np.asarray(bias, dtype=np.float32)
    B = x.shape[0]
    x1 = x.reshape(B, 16, 64, 2, 64, 2).transpose(0, 1, 3, 5, 2, 4)
    x2 = np.ascontiguousarray(x1).reshape(B, C1, N)
    xs = np.ascontiguousarray(x2[:, :, SAMPLE_FLAT])

    x2e = np.empty((B, 65, N), np.float32)
    x2e[:, :64] = x2
    x2e[:, 64] = 1.0
    xse = np.zeros((B, 66, M), np.float32)
    xse[:, :64] = xs * np.float32(2.0)
    xse[:, 65] = 1.0
    # padded o''-permutation: col o'' = j*32 + oq holds W row oq*4+j
    opp = np.arange(128)
    jj, oq = opp // 32, opp % 32
    valid = oq < 16
    orow = np.where(valid, (oq % 16) * 4 + jj, 0)
    wke = np.zeros((K, 66, 128), np.float32)
    for k in range(K):
        wke[k, :64] = np.where(valid[None, :],
                               weight[orow, :, k].T * np.float32(0.5), 0.0)
        wke[k, 65] = np.where(valid, bias[orow] * np.float32(1.0 / 3.0), 0.0)
    # delta fold: dle[b][o''][p] = (o''%32 == p%16) for p in b-block
    dle = np.zeros((B_PER_CORE, 128, 128), ml_dtypes.bfloat16)
    pp = np.arange(128)
    eq = (opp[:, None] % 32) == (pp[None, :] % 16)
    for b in range(B_PER_CORE):
        blk = (pp >= b * 32) & (pp < (b + 1) * 32)
        dle[b] = (eq & blk[None, :]).astype(ml_dtypes.bfloat16)

    in_maps = []
    for core in range(N_CORES):
        sl = slice(core * B_PER_CORE, (core + 1) * B_PER_CORE)
        in_maps.append({
            "x2e": np.ascontiguousarray(x2e[sl]),
            "xse": np.ascontiguousarray(xse[sl]),
            "wke": wke,
            "dle": dle,
        })
    return in_maps


def host_post(results):
    """Per-core outd [128, 8192] -> full output [32, 16, 128, 128]."""
    B = N_CORES * B_PER_CORE
    out = np.empty((B, C1, N), np.float32)
    for core in range(N_CORES):
        o = results[core]["outd"]  # [128, 8192] f32
        # partition = (b, th, oq); free = (s, pg, cp, pl, j); o = oq*4+j
        arr = o.reshape(B_PER_CORE, 2, 16, NSPLIT, 8, CPS, 16, 4)
        # out[b, oq*4+j, th*2048 + (s*CPS+cp)*128 + pg*16 + pl]
        blk = (arr.transpose(0, 2, 7, 1, 3, 5, 4, 6)  # b oq j th s cp pg pl
               .reshape(B_PER_CORE, C1, N))
        out[core * B_PER_CORE:(core + 1) * B_PER_CORE] = blk
    out = out.reshape(B, C1, 64, 64)
    y = (out.reshape(B, 16, 2, 2, 64, 64).transpose(0, 1, 4, 2, 5, 3)
         .reshape(B, 16, 128, 128))
    return np.ascontiguousarray(y)


def kernel(x, weight, bias):
    if "nc" not in _CACHE:
        _CACHE["nc"] = build_program()
    nc = _CACHE["nc"]
    in_maps = host_prep(x, weight, bias)
    res = run_bass_kernel_spmd(nc, in_maps, core_ids=list(range(N_CORES)))
    return host_post(res.results)


# revision 15
# speedup vs baseline: 1.6639x; 1.0031x over previous
"""Trainium2 Bass kernel for nn_Conv2d_NN_spatial (retrieval_knn).

Full-input contract: kernel(**inputs) takes the unsharded inputs and returns
the full output. Internally: data-parallel over batch across 8 NeuronCores
(4 batches per core).

Per-core algorithm:
  1. negd2 = 2*dot(x2, xs) - m2  via one 65-row-contraction fp32 matmul per
     128-token chunk (n2 term omitted: constant per token, rank-invariant).
  2. top-3 neighbors per token via DVE max (top-8) + max_index directly on
     the PSUM distance tile (tie-break == jax top_k).
  3. Projected tables P_k = W_k @ xs + bias/3 (o-permuted padded weights),
     spread into j-slices of R and folded 4->1 across partitions by a
     delta-matrix matmul: TABQ holds, per partition (b,oquad,tokhalf), rows
     [(k,m), 4 o-values] bf16 -- no table DMA.
  4. GPSIMD ap_gather with d=4 fetches 4 o-channels per index, 6144 indices
     per gpsimd core, SPLIT into 4 quarter-gathers pipelined against the
     distance/top-k loop (split-major processing order).  Idx streams
     round-trip DRAM in [lane][word] layout so fold DMAs stay coarse.
  5. Sum the 3 gathered projections per split (2 DVE strided adds) -> out.
Pixel unshuffle/shuffle are pure layout transforms done host-side.
"""
import numpy as np
import ml_dtypes

import concourse.bacc as bacc
import concourse.bass as bass
import concourse.mybir as mybir
import concourse.tile as tile
from concourse.bass_utils import run_bass_kernel_spmd

F32 = mybir.dt.float32
BF16 = mybir.dt.bfloat16
U16 = mybir.dt.uint16
I16 = mybir.dt.int16

N_CORES = 8
B_PER_CORE = 4
C1 = 64          # unshuffled channels
N = 4096         # tokens per batch (64*64)
NHALF = N // 2   # tokens per gather group (th = token half)
M = 256          # samples
K = 3
NIDX = NHALF * K  # gather indices per gpsimd core (6144)
NW = NIDX // 16   # idx words per lane (384)
NSPLIT = 8        # pipelined gather splits
CPS = 16 // NSPLIT  # chunks-per-half per split (4)
WPS = NW // NSPLIT  # idx words per split (96)
IPS = NIDX // NSPLIT  # idx per split per core (1536)

SIDX = [0, 4, 8, 13, 17, 21, 25, 29, 34, 38, 42, 46, 50, 55, 59, 63]
SAMPLE_FLAT = (np.array(SIDX)[:, None] * 64 + np.array(SIDX)[None, :]).reshape(-1)

_CACHE = {}


def build_program():
    """Build the per-core Bass program (SPMD: same program on all 8 cores)."""
    nc = bacc.Bacc("TRN2", target_bir_lowering=False, debug=False,
                   enable_asserts=False)

    x2e = nc.dram_tensor("x2e", [B_PER_CORE, 65, N], F32, kind="ExternalInput")
    xse = nc.dram_tensor("xse", [B_PER_CORE, 66, M], F32, kind="ExternalInput")
    wke = nc.dram_tensor("wke", [K, 66, 128], F32, kind="ExternalInput")
    # delta fold matrices: [b][o'' 128][psum col 128] bf16
    dle = nc.dram_tensor("dle", [B_PER_CORE, 128, 128], BF16,
                         kind="ExternalInput")
    outd = nc.dram_tensor("outd", [128, 2 * N], F32, kind="ExternalOutput")
    # idx stream scratch in [lane][word] layout:
    # stream step i = ((cp*8+pg)*3 + k)*16 + pl -> lane pl, word cp*24+pg*3+k
    idxscr = nc.dram_tensor("idxscr", [NSPLIT, B_PER_CORE, 128, 2 * CPS * K],
                            U16, kind="Internal")

    AA = mybir.AluOpType

    with tile.TileContext(nc) as tc:
        with (
            tc.tile_pool(name="xp", bufs=3) as xp,
            tc.tile_pool(name="sp", bufs=4) as sp,
            tc.tile_pool(name="cst", bufs=1) as cst,
            tc.tile_pool(name="tbp", bufs=2) as tbp,
            tc.tile_pool(name="m8p", bufs=4) as m8p,
            tc.tile_pool(name="ixp", bufs=3) as ixp,
            tc.tile_pool(name="gp", bufs=1) as gp,
            tc.tile_pool(name="op", bufs=2) as op_,
            tc.tile_pool(name="ps", bufs=4, space=bass.MemorySpace.PSUM) as psp,
            tc.tile_pool(name="ps2", bufs=2, space=bass.MemorySpace.PSUM) as psp2,
            tc.tile_pool(name="ps3", bufs=2, space=bass.MemorySpace.PSUM) as psp3,
        ):
            onescol = cst.tile([64, 1], F32, tag="ones")
            nc.vector.memset(onescol[:], 1.0)
            wk = []
            for k in range(K):
                t = cst.tile([66, 128], F32, tag=f"wk{k}")
                nc.sync.dma_start(t[:], wke[k])
                wk.append(t)
            dl = []
            for b in range(B_PER_CORE):
                t = cst.tile([128, 128], BF16, tag=f"dl{b}")
                nc.sync.dma_start(t[:], dle[b])
                dl.append(t)

            # gather table: partition (b*32 + th*16 + oq), free (k, m, j)
            TABQ = gp.tile([128, K * M * 4], BF16, tag="TABQ")

            # ---- load S, compute m2 row, project + fold tables ----
            S_tiles = []
            for b in range(B_PER_CORE):
                S = sp.tile([66, M], F32, tag="S")
                nc.sync.dma_start(S[:], xse[b])
                SQ = sp.tile([64, M], F32, tag="SQ")
                nc.vector.tensor_tensor(SQ[:], S[0:64, :], S[0:64, :], op=AA.mult)
                m2ps = psp2.tile([128, M], F32, tag="tabps")
                nc.tensor.matmul(m2ps[64:65, :], onescol[:], SQ[:],
                                 tile_position=(0, 64))
                nc.scalar.activation(S[64:65, :], m2ps[64:65, :],
                                     mybir.ActivationFunctionType.Identity,
                                     bias=0.0, scale=-0.25)
                S_tiles.append(S)
                # P''-stack [128 o''=(j*32+oq), (k, m)] bf16
                PS = tbp.tile([128, K * M], BF16, tag="PS")
                for k in range(K):
                    tp = psp2.tile([128, M], F32, tag="tabps")
                    nc.tensor.matmul(tp[:], wk[k][:], S[:])
                    nc.vector.tensor_scalar_add(PS[:, k * M:(k + 1) * M],
                                                tp[:], 0.0)
                # R[o'', (k, m, j)] = PS[o'', (k,m)] iff j == o''//32 else 0
                R = tbp.tile([128, K * M * 4], BF16, tag="R")
                nc.vector.memset(R[:], 0.0)
                rv = R[:].rearrange("p (r j) -> p r j", j=4)
                for j in range(4):
                    nc.vector.tensor_scalar_add(
                        rv[j * 32:j * 32 + 16, :, j:j + 1],
                        PS[j * 32:j * 32 + 16, :].rearrange(
                            "p (r u) -> p r u", u=1), 0.0)
                # fold 4->1 partitions: TABQ[b*32 + th*16 + oq] = P[oq*4+j]
                for ch in range(6):
                    tq = psp3.tile([128, 512], F32, tag="tqps")
                    nc.tensor.matmul(tq[:], dl[b][:],
                                     R[:, ch * 512:(ch + 1) * 512])
                    nc.scalar.copy(
                        TABQ[b * 32:(b + 1) * 32, ch * 512:(ch + 1) * 512],
                        tq[b * 32:(b + 1) * 32, :])

            # ---- distance + top-3, split-major for gather pipelining ----
            G = gp.tile([128, NIDX * 4], BF16, tag="G")
            WIDX = cst.tile([128, NW], I16, tag="WIDX")
            for s in range(NSPLIT):
                for b in range(B_PER_CORE):
                    # chunks of this split: cp in [CPS*s, CPS*(s+1)) per half
                    X = xp.tile([65, 2 * CPS * 128], F32, tag="X")
                    xq = nc.sync if s == 0 else nc.scalar
                    for th in range(2):
                        xq.dma_start(
                            X[:, th * CPS * 128:(th + 1) * CPS * 128],
                            x2e[b][:, (th * 16 + CPS * s) * 128:
                                   (th * 16 + CPS * s + CPS) * 128])
                    S = S_tiles[b]
                    IDXS = ixp.tile([128, 2 * CPS * 8], U16, tag="IDXS")
                    for ci in range(2 * CPS):
                        nd = psp.tile([128, M], F32, tag="nd")
                        nc.tensor.matmul(nd[:], X[:, ci * 128:(ci + 1) * 128],
                                         S[0:65, :])
                        M8 = m8p.tile([128, 8], F32, tag="M8")
                        nc.vector.max(M8[:], nd[:])
                        nc.vector.max_index(IDXS[:, ci * 8:(ci + 1) * 8],
                                            M8[:], nd[:])

                    # slice k<3, add 256*k table-row offset
                    IDXC = ixp.tile([128, 2 * CPS * K], U16, tag="IDXC")
                    src = IDXS[:].rearrange("p (c e) -> p c e", e=8)
                    dst = IDXC[:].rearrange("p (c e) -> p c e", e=3)
                    for k in range(K):
                        nc.vector.tensor_scalar_add(dst[:, :, k:k + 1],
                                                    src[:, :, k:k + 1], 256 * k)
                    # fold hop 1: contiguous dump [128, 2*CPS*K] -> DRAM
                    nc.sync.dma_start(idxscr[s, b], IDXC[:])

                # fold hop 2: scramble on read into wrapped [lane][word]
                # word (pg, cp, k); WIDX[16g+q, f] = stream i = f*16 + q
                for b in range(B_PER_CORE):
                    srcw = idxscr[s, b].rearrange(
                        "(pg q) (th ck) -> q th pg ck",
                        pg=8, q=16, th=2, ck=CPS * K)
                    for th in range(2):
                        g = b * 2 + th
                        dst = WIDX[g * 16:(g + 1) * 16,
                                   WPS * s:WPS * (s + 1)].bitcast(U16).rearrange(
                            "q (pg ck) -> q pg ck", pg=8, ck=CPS * K)
                        nc.sync.dma_start(dst, srcw[:, th])

                # quarter-gather: all 4 batches, d=4 o-channels per index
                nc.gpsimd.ap_gather(
                    G[:, IPS * 4 * s:IPS * 4 * (s + 1)].rearrange(
                        "p (i j) -> p i j", j=4),
                    TABQ[:].rearrange("p (r j) -> p r j", j=4),
                    WIDX[:, WPS * s:WPS * (s + 1)],
                    channels=128, num_elems=K * M, d=4, num_idxs=IPS)

            # ---- k-sum per split: OUT[p,(cp,pg,pl,j)] = sum_k G[...] ----
            for s in range(NSPLIT):
                gv = G[:, IPS * 4 * s:IPS * 4 * (s + 1)].rearrange(
                    "p (pg cp k pl j) -> p (pg cp) k (pl j)",
                    pg=8, cp=CPS, k=K, pl=16, j=4)
                T1 = op_.tile([128, CPS * 8 * 64], F32, tag="T1")
                t1v = T1[:].rearrange("p (t w) -> p t w", w=64)
                OUT = op_.tile([128, CPS * 8 * 64], F32, tag="OUT")
                ov = OUT[:].rearrange("p (t w) -> p t w", w=64)
                nc.vector.tensor_tensor(t1v, gv[:, :, 0, :], gv[:, :, 1, :],
                                        op=AA.add)
                nc.vector.tensor_tensor(ov, t1v, gv[:, :, 2, :], op=AA.add)
                nc.sync.dma_start(
                    outd[:, CPS * 8 * 64 * s:CPS * 8 * 64 * (s + 1)], OUT[:])

    nc.compile()
    return nc


def host_prep(x, weight, bias):
    """Full inputs -> per-core in_maps (list of 8 dicts)."""
    x = np.ascontiguousarray(np.asarray(x), dtype=np.float32)
    weight = np.asarray(weight, dtype=np.float32)
    bias = np.asarray(bias, dtype=np.float32)
    B = x.shape[0]
    x1 = x.reshape(B, 16, 64, 2, 64, 2).transpose(0, 1, 3, 5, 2, 4)
    x2 = np.ascontiguousarray(x1).reshape(B, C1, N)
    xs = np.ascontiguousarray(x2[:, :, SAMPLE_FLAT])

    x2e = np.empty((B, 65, N), np.float32)
    x2e[:, :64] = x2
    x2e[:, 64] = 1.0
    xse = np.zeros((B, 66, M), np.float32)
    xse[:, :64] = xs * np.float32(2.0)
    xse[:, 65] = 1.0
    # padded o''-permutation: col o'' = j*32 + oq holds W row oq*4+j
    opp = np.arange(128)
    jj, oq = opp // 32, opp % 32
    valid = oq < 16
    orow = np.where(valid, (oq % 16) * 4 + jj, 0)
    wke = np.zeros((K, 66, 128), np.float32)
    for k in range(K):
        wke[k, :64] = np.where(valid[None, :],
                               weight[orow, :, k].T * np.float32(0.5), 0.0)
        wke[k, 65] = np.where(valid, bias[orow] * np.float32(1.0 / 3.0), 0.0)
    # delta fold: dle[b][o''][p] = (o''%32 == p%16) for p in b-block
    dle = np.zeros((B_PER_CORE, 128, 128), ml_dtypes.bfloat16)
    pp = np.arange(128)
    eq = (opp[:, None] % 32) == (pp[None, :] % 16)
    for b in range(B_PER_CORE):
        blk = (pp >= b * 32) & (pp < (b + 1) * 32)
        dle[b] = (eq & blk[None, :]).astype(ml_dtypes.bfloat16)

    in_maps = []
    for core in range(N_CORES):
        sl = slice(core * B_PER_CORE, (core + 1) * B_PER_CORE)
        in_maps.append({
            "x2e": np.ascontiguousarray(x2e[sl]),
            "xse": np.ascontiguousarray(xse[sl]),
            "wke": wke,
            "dle": dle,
        })
    return in_maps


def host_post(results):
    """Per-core outd [128, 8192] -> full output [32, 16, 128, 128]."""
    B = N_CORES * B_PER_CORE
    out = np.empty((B, C1, N), np.float32)
    for core in range(N_CORES):
        o = results[core]["outd"]  # [128, 8192] f32
        # partition = (b, th, oq); free = (s, pg, cp, pl, j); o = oq*4+j
        arr = o.reshape(B_PER_CORE, 2, 16, NSPLIT, 8, CPS, 16, 4)
        # out[b, oq*4+j, th*2048 + (s*CPS+cp)*128 + pg*16 + pl]
        blk = (arr.transpose(0, 2, 7, 1, 3, 5, 4, 6)  # b oq j th s cp pg pl
               .reshape(B_PER_CORE, C1, N))
        out[core * B_PER_CORE:(core + 1) * B_PER_CORE] = blk
    out = out.reshape(B, C1, 64, 64)
    y = (out.reshape(B, 16, 2, 2, 64, 64).transpose(0, 1, 4, 2, 5, 3)
         .reshape(B, 16, 128, 128))
    return np.ascontiguousarray(y)


def kernel(x, weight, bias):
    if "nc" not in _CACHE:
        _CACHE["nc"] = build_program()
    nc = _CACHE["nc"]
    in_maps = host_prep(x, weight, bias)
    res = run_bass_kernel_spmd(nc, in_maps, core_ids=list(range(N_CORES)))
    return host_post(res.results)
